# revision 25
# baseline (speedup 1.0000x reference)
"""Bass/Trainium2 kernel for nn_DreamAttention (dense transformer attention,
dead-softmax variant).

Math (per reference): q/k/v linear projections + RoPE, scores = q @ k^T /
sqrt(HD) (softmax computed but DISCARDED in the source), out = (scores @ v)
@ Wo^T.

Because no softmax is applied, attention is linear:
    (q @ k^T) @ v == q @ (k^T @ v)
so we compute the tiny per-head Gram matrix KV = k^T v  [HD, HD] instead of
the S x S score matrix (16x fewer FLOPs, no S x S materialization).

The q-side RoPE is folded into the attention matmul (RoPE is linear):
    attn_h = KV_h^T (cos*q_h) + KVp_h^T (sin* * q_h)
where KVp is KV with its partition halves swapped and sin* carries the
rotate-half signs. This lets the q projection emit feature-major tiles
directly (weight-stationary matmul), avoiding a transpose stage.

Sharding: data-parallel over tokens. 8 cores x 512 tokens (cores 0-3 hold
batch 0, cores 4-7 batch 1). Each core computes q/k/v for its tokens
(weights replicated), partial per-head KV over its tokens, an AllReduce of
the 1MB KV block within each 4-core batch group (overlapped with the q
projection), then attn and the output projection for its tokens. The scale
1/sqrt(HD) is folded into k's RoPE tables on the host.

Matmuls run in float32r (fp32 data, single-pass PE mode: full rate at free
dim >= 256 vs 4x slower true fp32). DMA triggers are spread across the SP
and ACT sequencers to keep trigger issue off the critical path.
"""

import math
from contextlib import ExitStack

import numpy as np

import concourse.bass as bass
import concourse.mybir as mybir
import concourse.tile as tile
from concourse import bacc
from concourse import bass_utils

P = 128
HD = 128
F32 = mybir.dt.float32
F32R = mybir.dt.float32r


def ts(i, size):
    return slice(i * size, (i + 1) * size)


def emit_attn(tc, ctx, io, t_core, d_model, replica_groups):
    """Emit the per-core attention kernel.

    io: DRAM APs: xT [d_model, t_core]; wqT/wkT/wvT/woT [d_model, d_model];
    bkb/bvb [128, d_model] (broadcast biases); bqd [128, d_model/128]
    (bq in feature-major per-tile columns); cosk/sinkf [t_core, d_model]
    (token-major k tables, sign-folded + 1/sqrt(HD) prescaled);
    cosqD/sinqD [128, t_core] (feature-major q tables, sinqD sign-folded);
    y [t_core, d_model].
    """
    nc = tc.nc
    T_TILES = t_core // P
    DIN = d_model // P          # number of 128-wide feature tiles
    NH = d_model // HD          # heads
    N_CHUNK = min(512, d_model)
    CHUNKS = d_model // N_CHUNK
    PAIR = 2 if CHUNKS % 2 == 0 else 1   # chunks per W load / psum round
    QG = min(8, DIN)            # q-proj feature tiles per psum round

    sb = ctx.enter_context(tc.tile_pool(name="sb", bufs=1))
    ps = ctx.enter_context(tc.tile_pool(name="ps", bufs=8, space="PSUM"))
    dram = ctx.enter_context(tc.tile_pool(name="dram", bufs=2, space="DRAM"))

    def b2(name, dtype=F32R, width=None):
        return sb.tile([P, width or min(512, t_core)], dtype, name=name,
                       tag="b2", bufs=38)

    def b4(name, dtype=F32R):
        return sb.tile([P, PAIR * N_CHUNK], dtype, name=name, tag="b4", bufs=6)

    def b8(name, dtype=F32R):
        return sb.tile([P, d_model], dtype, name=name, tag="b8", bufs=11)

    def psum(name, width, dtype=F32):
        return ps.tile([P, width], dtype, name=name, tag="ps", bufs=8)

    # Resident x^T tiles [din, t] (matmul operand for all three projections)
    xt_tiles = []
    for din in range(DIN):
        xt = sb.tile([P, t_core], F32R, name=f"xt{din}", tag="b2", bufs=38)
        nc.scalar.dma_start(xt[:], io["xT"][ts(din, P), :])
        xt_tiles.append(xt)

    def project_tmajor(wT_ap, bias_ap, out_tiles, dma_eng, pairs=None):
        """out[t, dout] = x @ W^T + b, token-major tiles [128, d_model]."""
        for pair in (range(CHUNKS // PAIR) if pairs is None else pairs):
            psums = [psum(f"pp{i}", N_CHUNK) for i in range(T_TILES * PAIR)]
            for din in range(DIN):
                wt = b4(f"w{din}")
                dma_eng.dma_start(wt[:], wT_ap[ts(din, P),
                                               ts(pair, PAIR * N_CHUNK)])
                for t in range(T_TILES):
                    for p in range(PAIR):
                        nc.tensor.matmul(
                            psums[t * PAIR + p][:],
                            xt_tiles[din][:, ts(t, P)],
                            wt[:, ts(p, N_CHUNK)],
                            start=(din == 0),
                            stop=(din == DIN - 1),
                        )
            bt = b4(f"bias_p{pair}")
            dma_eng.dma_start(bt[:], bias_ap[:, ts(pair, PAIR * N_CHUNK)])
            for t in range(T_TILES):
                for p in range(PAIR):
                    chunk = pair * PAIR + p
                    nc.vector.tensor_add(
                        out_tiles[t][:, ts(chunk, N_CHUNK)],
                        psums[t * PAIR + p][:],
                        bt[:, ts(p, N_CHUNK)],
                    )

    def rope_tmajor(tiles, cos_ap, sinf_ap, t_range=None):
        """In-place RoPE on token-major tiles using compact [t, HD] tables
        broadcast across heads.

        out = x*cos + rot_half(x)*sin; sinf is sign-folded so
        rot_half(x)*sin == gather(x, +-64) * sinf elementwise.
        """
        h2 = HD // 2

        def bc(ap2d):  # [128, w] -> [128, NH, w] broadcast view
            return ap2d.unsqueeze(1).broadcast_to([P, NH, ap2d.shape[-1]])

        for t in (range(T_TILES) if t_range is None else t_range):
            ct = b2(f"cos{t}", width=HD)
            st = b2(f"sin{t}", width=HD)
            nc.scalar.dma_start(ct[:], cos_ap[ts(t, P), :])
            nc.scalar.dma_start(st[:], sinf_ap[ts(t, P), :])
            tmp = b8(f"ropetmp{t}")
            x3 = tiles[t][:].rearrange("p (h d) -> p h d", d=HD)
            t3 = tmp[:].rearrange("p (h d) -> p h d", d=HD)
            nc.vector.tensor_mul(t3[:, :, 0:h2], x3[:, :, h2:HD],
                                 bc(st[:, 0:h2]))
            nc.vector.tensor_mul(t3[:, :, h2:HD], x3[:, :, 0:h2],
                                 bc(st[:, h2:HD]))
            nc.vector.tensor_mul(x3, x3, bc(ct[:]))
            nc.vector.tensor_add(tiles[t][:], tiles[t][:], tmp[:])

    # ---- K/V projections + RoPE(k) ----
    k_tiles = [b8(f"k{t}") for t in range(T_TILES)]
    project_tmajor(io["wkT"], io["bkb"], k_tiles, nc.sync)
    rope_tmajor(k_tiles, io["cosk"], io["sinkf"])
    v_tiles = [b8(f"v{t}") for t in range(T_TILES)]
    project_tmajor(io["wvT"], io["bvb"], v_tiles, nc.scalar)

    # ---- Per-head Gram matrices KV[h] = k_h^T @ v_h (partial over this
    # core's tokens), packed [128, NH*HD]. The moving operand spans two
    # heads (N=256) to stay at full fp32r rate; the unwanted half of each
    # product is discarded at eviction. ----
    kv_sb = b8("kvsb")
    for h in range(NH):
        base = min(h, NH - 2) if NH >= 2 else 0
        good = h - base
        width = min(2 * HD, (NH - base) * HD)
        kvp = psum(f"kvp{h}", width)
        for t in range(T_TILES):
            nc.tensor.matmul(
                kvp[:],
                k_tiles[t][:, ts(h, HD)],
                v_tiles[t][:, base * HD: base * HD + width],
                start=(t == 0),
                stop=(t == T_TILES - 1),
            )
        nc.vector.tensor_copy(kv_sb[:, ts(h, HD)], kvp[:, ts(good, HD)])

    # ---- AllReduce the KV partials within the batch group ----
    kv_in = dram.tile([P, NH * HD], F32R, name="kv_in")
    kv_out = dram.tile([P, NH * HD], F32R, name="kv_out")
    nc.gpsimd.dma_start(kv_in[:], kv_sb[:])
    nc.gpsimd.collective_compute(
        "AllReduce",
        mybir.AluOpType.add,
        replica_groups=replica_groups,
        ins=[kv_in.opt()],
        outs=[kv_out.opt()],
    )
    kv_red = b8("kvred")
    nc.gpsimd.dma_start(kv_red[:], kv_out[:])
    # Partition-half-swapped copy for the folded q-side RoPE
    kv_perm = b8("kvperm")
    h2 = HD // 2
    nc.gpsimd.dma_start(kv_perm[0:h2, :], kv_out[h2:HD, :])
    nc.gpsimd.dma_start(kv_perm[h2:HD, :], kv_out[0:h2, :])

    # ---- Q projection, feature-major: qD[dout, t] = W q-row blocks ----
    qd_tiles = [None] * DIN
    bqd_sb = b2("bqd", dtype=F32, width=DIN)
    nc.sync.dma_start(bqd_sb[:], io["bqd"][:])
    for g in range(DIN // QG):
        psums = [psum(f"qp{i}", t_core) for i in range(QG)]
        for din in range(DIN):
            wt = b4(f"wq{din}")
            nc.sync.dma_start(wt[:], io["wqT"][ts(din, P),
                                               ts(g, QG * P)])
            for j in range(QG):
                nc.tensor.matmul(
                    psums[j][:],
                    wt[:, ts(j, P)],
                    xt_tiles[din][:],
                    start=(din == 0),
                    stop=(din == DIN - 1),
                )
        for j in range(QG):
            dout = g * QG + j
            qd = b2(f"qd{dout}", width=t_core)
            nc.vector.tensor_scalar_add(qd[:], psums[j][:],
                                        bqd_sb[:, dout:dout + 1])
            qd_tiles[dout] = qd

    cosq = b2("cosq", width=t_core)
    sinq = b2("sinq", width=t_core)
    nc.scalar.dma_start(cosq[:], io["cosqD"][:])
    nc.scalar.dma_start(sinq[:], io["sinqD"][:])

    # ---- attn_h[d2, t] = KV_h^T (cos*q_h) + KVp_h^T (sin* q_h) ----
    attn_tiles = []
    for h in range(NH):
        qc = b2(f"qc{h}", width=t_core)
        nc.vector.tensor_mul(qc[:], qd_tiles[h][:], cosq[:])
        qs = b2(f"qs{h}", width=t_core)
        nc.vector.tensor_mul(qs[:], qd_tiles[h][:], sinq[:])
        ap = psum(f"ap{h}", t_core)
        nc.tensor.matmul(ap[:], kv_red[:, ts(h, HD)], qc[:],
                         start=True, stop=False)
        nc.tensor.matmul(ap[:], kv_perm[:, ts(h, HD)], qs[:],
                         start=False, stop=True)
        asb = b2(f"asb{h}", width=t_core)
        nc.vector.tensor_copy(asb[:], ap[:])
        attn_tiles.append(asb)

    # ---- Output projection: y[t, dout] = attn @ Wo^T, token-major ----
    for pair in range(CHUNKS // PAIR):
        psums = [psum(f"op{i}", N_CHUNK) for i in range(T_TILES * PAIR)]
        for dmid in range(DIN):
            wt = b4(f"wo{dmid}")
            nc.scalar.dma_start(wt[:], io["woT"][ts(dmid, P),
                                                 ts(pair, PAIR * N_CHUNK)])
            for t in range(T_TILES):
                for p in range(PAIR):
                    nc.tensor.matmul(
                        psums[t * PAIR + p][:],
                        attn_tiles[dmid][:, ts(t, P)],
                        wt[:, ts(p, N_CHUNK)],
                        start=(dmid == 0),
                        stop=(dmid == DIN - 1),
                    )
        for t in range(T_TILES):
            for p in range(PAIR):
                chunk = pair * PAIR + p
                osb = b2(f"osb{chunk}_{t}", dtype=F32)
                nc.vector.tensor_copy(osb[:], psums[t * PAIR + p][:])
                nc.gpsimd.dma_start(io["y"][ts(t, P), ts(chunk, N_CHUNK)],
                                    osb[:])


def build_nc(t_core, d_model, num_devices, replica_groups, reps=1):
    nc = bacc.Bacc("TRN2", target_bir_lowering=False, debug=False,
                   num_devices=num_devices)
    io = {}
    io["xT"] = nc.dram_tensor("xT", [d_model, t_core], F32R,
                              kind="ExternalInput").ap()
    for nm in ("wqT", "wkT", "wvT", "woT"):
        io[nm] = nc.dram_tensor(nm, [d_model, d_model], F32R,
                                kind="ExternalInput").ap()
    for nm in ("bkb", "bvb"):
        io[nm] = nc.dram_tensor(nm, [P, d_model], F32R,
                                kind="ExternalInput").ap()
    io["bqd"] = nc.dram_tensor("bqd", [P, d_model // P], F32,
                               kind="ExternalInput").ap()
    for nm in ("cosk", "sinkf"):
        io[nm] = nc.dram_tensor(nm, [t_core, HD], F32R,
                                kind="ExternalInput").ap()
    for nm in ("cosqD", "sinqD"):
        io[nm] = nc.dram_tensor(nm, [P, t_core], F32R,
                                kind="ExternalInput").ap()
    io["y"] = nc.dram_tensor("y", [t_core, d_model], F32,
                             kind="ExternalOutput").ap()

    with tile.TileContext(nc) as tc:
        for _ in range(reps):
            with ExitStack() as ctx:
                emit_attn(tc, ctx, io, t_core, d_model, replica_groups)
    nc.compile()
    return nc


# ---------------- host side ----------------

B, S, D = 2, 2048, 2048
NH_FULL = 16
MAX_POS = 4096
ROPE_THETA = 10000.0
N_CORES = 8
T_CORE = B * S // N_CORES

_cache = {}


def _rope_tables():
    inv_freq = (np.float32(1.0) /
                np.power(np.float32(ROPE_THETA),
                         np.arange(0, HD, 2, dtype=np.float32) /
                         np.float32(HD))).astype(np.float32)
    t = np.arange(MAX_POS, dtype=np.float32)
    freqs = np.outer(t, inv_freq).astype(np.float32)
    emb = np.concatenate((freqs, freqs), axis=-1)
    return np.cos(emb).astype(np.float32), np.sin(emb).astype(np.float32)


def _get_nc():
    if "nc" not in _cache:
        _cache["nc"] = build_nc(T_CORE, D, N_CORES,
                                [[0, 1, 2, 3], [4, 5, 6, 7]])
    return _cache["nc"]


def _host_inputs(hidden_states, position_ids, Wq, bq, Wk, bk, Wv, bv, Wo):
    x = np.asarray(hidden_states, dtype=np.float32).reshape(B * S, D)
    pos = np.asarray(position_ids).astype(np.int64).reshape(B * S)

    cos_t, sin_t = _rope_tables()
    cos = cos_t[pos]            # [B*S, HD]
    sin = sin_t[pos]
    # token-major k tables: sign-folded sin + 1/sqrt(HD) fold
    sinf = sin.copy()
    sinf[:, : HD // 2] *= np.float32(-1.0)
    scale = np.float32(1.0 / math.sqrt(HD))
    # feature-major q tables: sin* = +sin (i<64), -sin (i>=64)
    sinq = sin.copy()
    sinq[:, HD // 2:] *= np.float32(-1.0)

    wqT = np.ascontiguousarray(np.asarray(Wq, np.float32).T)
    wkT = np.ascontiguousarray(np.asarray(Wk, np.float32).T)
    wvT = np.ascontiguousarray(np.asarray(Wv, np.float32).T)
    woT = np.ascontiguousarray(np.asarray(Wo, np.float32).T)
    bkb = np.ascontiguousarray(np.broadcast_to(np.asarray(bk, np.float32), (P, D)))
    bvb = np.ascontiguousarray(np.broadcast_to(np.asarray(bv, np.float32), (P, D)))
    bqd = np.ascontiguousarray(np.asarray(bq, np.float32).reshape(D // P, P).T)

    in_maps = []
    for c in range(N_CORES):
        sl = slice(c * T_CORE, (c + 1) * T_CORE)
        in_maps.append({
            "xT": np.ascontiguousarray(x[sl].T),
            "wqT": wqT, "wkT": wkT, "wvT": wvT, "woT": woT,
            "bkb": bkb, "bvb": bvb, "bqd": bqd,
            "cosk": np.ascontiguousarray(cos[sl] * scale),
            "sinkf": np.ascontiguousarray(sinf[sl] * scale),
            "cosqD": np.ascontiguousarray(cos[sl].T),
            "sinqD": np.ascontiguousarray(sinq[sl].T),
        })
    return in_maps


def kernel(hidden_states, position_ids, Wq, bq, Wk, bk, Wv, bv, Wo):
    in_maps = _host_inputs(hidden_states, position_ids,
                           Wq, bq, Wk, bk, Wv, bv, Wo)
    nc = _get_nc()
    res = bass_utils.run_bass_kernel_spmd(nc, in_maps,
                                          core_ids=list(range(N_CORES)))
    out = np.concatenate([res.results[c]["y"] for c in range(N_CORES)], axis=0)
    return out.reshape(B, S, D)


# revision 27
# speedup vs baseline: 1.0538x; 1.0538x over previous
"""Bass/Trainium2 kernel for nn_DreamAttention (dense transformer attention,
dead-softmax variant).

Math (per reference): q/k/v linear projections + RoPE, scores = q @ k^T /
sqrt(HD) (softmax computed but DISCARDED in the source), out = (scores @ v)
@ Wo^T.

Because no softmax is applied, attention is linear:
    (q @ k^T) @ v == q @ (k^T @ v)
so we compute the tiny per-head Gram matrix KV = k^T v  [HD, HD] instead of
the S x S score matrix (16x fewer FLOPs, no S x S materialization).

The q-side RoPE is folded into the attention matmul (RoPE is linear):
    attn_h = KV_h^T (cos*q_h) + KVp_h^T (sin* * q_h)
where KVp is KV with its partition halves swapped and sin* carries the
rotate-half signs. This lets the q projection emit feature-major tiles
directly (weight-stationary matmul), avoiding a transpose stage.

Sharding: data-parallel over tokens. 8 cores x 512 tokens (cores 0-3 hold
batch 0, cores 4-7 batch 1). Each core computes q/k/v for its tokens
(weights replicated), partial per-head KV over its tokens, an AllReduce of
the 1MB KV block within each 4-core batch group (overlapped with the q
projection), then attn and the output projection for its tokens. The scale
1/sqrt(HD) is folded into k's RoPE tables on the host.

Matmuls run in float32r (fp32 data, single-pass PE mode: full rate at free
dim >= 256 vs 4x slower true fp32). DMA triggers are spread across the SP
and ACT sequencers to keep trigger issue off the critical path.
"""

import math
from contextlib import ExitStack

import numpy as np

import concourse.mybir as mybir
import concourse.tile as tile
from concourse import bacc
from concourse import bass_utils

P = 128
HD = 128
F32 = mybir.dt.float32
F32R = mybir.dt.float32r


def ts(i, size):
    return slice(i * size, (i + 1) * size)


def emit_attn(tc, ctx, io, t_core, d_model, replica_groups):
    """Emit the per-core attention kernel.

    io: DRAM APs: xT [d_model, t_core]; wqT/wkT/wvT/woT [d_model, d_model];
    bkb/bvb [128, d_model] (broadcast biases); bqd [128, d_model/128]
    (bq in feature-major per-tile columns); cosk/sinkf [t_core, d_model]
    (token-major k tables, sign-folded + 1/sqrt(HD) prescaled);
    cosqD/sinqD [128, t_core] (feature-major q tables, sinqD sign-folded);
    y [t_core, d_model].
    """
    nc = tc.nc
    T_TILES = t_core // P
    DIN = d_model // P          # number of 128-wide feature tiles
    NH = d_model // HD          # heads
    N_CHUNK = min(512, d_model)
    CHUNKS = d_model // N_CHUNK
    PAIR = 2 if CHUNKS % 2 == 0 else 1   # chunks per W load / psum round
    QG = min(8, DIN)            # q-proj feature tiles per psum round

    sb = ctx.enter_context(tc.tile_pool(name="sb", bufs=1))
    ps = ctx.enter_context(tc.tile_pool(name="ps", bufs=8, space="PSUM"))
    dram = ctx.enter_context(tc.tile_pool(name="dram", bufs=2, space="DRAM"))

    def b2(name, dtype=F32R, width=None):
        return sb.tile([P, width or min(512, t_core)], dtype, name=name,
                       tag="b2", bufs=38)

    def b4(name, dtype=F32R):
        return sb.tile([P, PAIR * N_CHUNK], dtype, name=name, tag="b4", bufs=6)

    def b8(name, dtype=F32R):
        return sb.tile([P, d_model], dtype, name=name, tag="b8", bufs=11)

    def psum(name, width, dtype=F32):
        return ps.tile([P, width], dtype, name=name, tag="ps", bufs=8)

    # Resident x^T tiles [din, t] (matmul operand for all three projections)
    xt_tiles = []
    for din in range(DIN):
        xt = sb.tile([P, t_core], F32R, name=f"xt{din}", tag="b2", bufs=38)
        nc.scalar.dma_start(xt[:], io["xT"][ts(din, P), :])
        xt_tiles.append(xt)

    def project_tmajor(wT_ap, bias_ap, out_tiles, dma_eng, pairs=None):
        """out[t, dout] = x @ W^T + b, token-major tiles [128, d_model]."""
        for pair in (range(CHUNKS // PAIR) if pairs is None else pairs):
            psums = [psum(f"pp{i}", N_CHUNK) for i in range(T_TILES * PAIR)]
            for din in range(DIN):
                wt = b4(f"w{din}")
                dma_eng.dma_start(wt[:], wT_ap[ts(din, P),
                                               ts(pair, PAIR * N_CHUNK)])
                for t in range(T_TILES):
                    for p in range(PAIR):
                        nc.tensor.matmul(
                            psums[t * PAIR + p][:],
                            xt_tiles[din][:, ts(t, P)],
                            wt[:, ts(p, N_CHUNK)],
                            start=(din == 0),
                            stop=(din == DIN - 1),
                        )
            bt = b4(f"bias_p{pair}")
            dma_eng.dma_start(bt[:], bias_ap[:, ts(pair, PAIR * N_CHUNK)])
            for t in range(T_TILES):
                for p in range(PAIR):
                    chunk = pair * PAIR + p
                    nc.vector.tensor_add(
                        out_tiles[t][:, ts(chunk, N_CHUNK)],
                        psums[t * PAIR + p][:],
                        bt[:, ts(p, N_CHUNK)],
                    )

    def rope_tmajor(tiles, cos_ap, sinf_ap, t_range=None):
        """In-place RoPE on token-major tiles using compact [t, HD] tables
        broadcast across heads.

        out = x*cos + rot_half(x)*sin; sinf is sign-folded so
        rot_half(x)*sin == gather(x, +-64) * sinf elementwise.
        """
        h2 = HD // 2

        def bc(ap2d):  # [128, w] -> [128, NH, w] broadcast view
            return ap2d.unsqueeze(1).broadcast_to([P, NH, ap2d.shape[-1]])

        for t in (range(T_TILES) if t_range is None else t_range):
            ct = b2(f"cos{t}", width=HD)
            st = b2(f"sin{t}", width=HD)
            nc.scalar.dma_start(ct[:], cos_ap[ts(t, P), :])
            nc.scalar.dma_start(st[:], sinf_ap[ts(t, P), :])
            tmp = b8(f"ropetmp{t}")
            x3 = tiles[t][:].rearrange("p (h d) -> p h d", d=HD)
            t3 = tmp[:].rearrange("p (h d) -> p h d", d=HD)
            nc.vector.tensor_mul(t3[:, :, 0:h2], x3[:, :, h2:HD],
                                 bc(st[:, 0:h2]))
            nc.vector.tensor_mul(t3[:, :, h2:HD], x3[:, :, 0:h2],
                                 bc(st[:, h2:HD]))
            nc.vector.tensor_mul(x3, x3, bc(ct[:]))
            nc.vector.tensor_add(tiles[t][:], tiles[t][:], tmp[:])

    # ---- K/V projections + RoPE(k) ----
    k_tiles = [b8(f"k{t}") for t in range(T_TILES)]
    project_tmajor(io["wkT"], io["bkb"], k_tiles, nc.sync)
    rope_tmajor(k_tiles, io["cosk"], io["sinkf"])
    v_tiles = [b8(f"v{t}") for t in range(T_TILES)]
    project_tmajor(io["wvT"], io["bvb"], v_tiles, nc.scalar)

    # ---- Per-head Gram matrices KV[h] = k_h^T @ v_h (partial over this
    # core's tokens), packed [128, NH*HD]. The moving operand spans two
    # heads (N=256) to stay at full fp32r rate; the unwanted half of each
    # product is discarded at eviction. ----
    kv_sb = b8("kvsb")
    for h in range(NH):
        base = min(h, NH - 2) if NH >= 2 else 0
        good = h - base
        width = min(2 * HD, (NH - base) * HD)
        kvp = psum(f"kvp{h}", width)
        for t in range(T_TILES):
            nc.tensor.matmul(
                kvp[:],
                k_tiles[t][:, ts(h, HD)],
                v_tiles[t][:, base * HD: base * HD + width],
                start=(t == 0),
                stop=(t == T_TILES - 1),
            )
        nc.vector.tensor_copy(kv_sb[:, ts(h, HD)], kvp[:, ts(good, HD)])

    # ---- AllReduce the KV partials within the batch group ----
    kv_in = dram.tile([P, NH * HD], F32R, name="kv_in")
    kv_out = dram.tile([P, NH * HD], F32R, name="kv_out")
    nc.gpsimd.dma_start(kv_in[:], kv_sb[:])
    nc.gpsimd.collective_compute(
        "AllReduce",
        mybir.AluOpType.add,
        replica_groups=replica_groups,
        ins=[kv_in.opt()],
        outs=[kv_out.opt()],
    )
    kv_red = b8("kvred")
    nc.gpsimd.dma_start(kv_red[:], kv_out[:])
    # Partition-half-swapped copy for the folded q-side RoPE
    kv_perm = b8("kvperm")
    h2 = HD // 2
    nc.gpsimd.dma_start(kv_perm[0:h2, :], kv_out[h2:HD, :])
    nc.gpsimd.dma_start(kv_perm[h2:HD, :], kv_out[0:h2, :])

    # ---- Q projection, feature-major: qD[dout, t] = W q-row blocks ----
    qd_tiles = [None] * DIN
    bqd_sb = b2("bqd", dtype=F32, width=DIN)
    nc.sync.dma_start(bqd_sb[:], io["bqd"][:])
    for g in range(DIN // QG):
        psums = [psum(f"qp{i}", t_core) for i in range(QG)]
        for din in range(DIN):
            wt = b4(f"wq{din}")
            nc.sync.dma_start(wt[:], io["wqT"][ts(din, P),
                                               ts(g, QG * P)])
            for j in range(QG):
                nc.tensor.matmul(
                    psums[j][:],
                    wt[:, ts(j, P)],
                    xt_tiles[din][:],
                    start=(din == 0),
                    stop=(din == DIN - 1),
                )
        for j in range(QG):
            dout = g * QG + j
            qd = b2(f"qd{dout}", width=t_core)
            nc.vector.tensor_scalar_add(qd[:], psums[j][:],
                                        bqd_sb[:, dout:dout + 1])
            qd_tiles[dout] = qd

    cosq = b2("cosq", width=t_core)
    sinq = b2("sinq", width=t_core)
    nc.scalar.dma_start(cosq[:], io["cosqD"][:])
    nc.scalar.dma_start(sinq[:], io["sinqD"][:])

    # ---- attn_h[d2, t] = KV_h^T (cos*q_h) + KVp_h^T (sin* q_h) ----
    attn_tiles = []
    for h in range(NH):
        qc = b2(f"qc{h}", width=t_core)
        nc.vector.tensor_mul(qc[:], qd_tiles[h][:], cosq[:])
        qs = b2(f"qs{h}", width=t_core)
        nc.vector.tensor_mul(qs[:], qd_tiles[h][:], sinq[:])
        ap = psum(f"ap{h}", t_core)
        nc.tensor.matmul(ap[:], kv_red[:, ts(h, HD)], qc[:],
                         start=True, stop=False)
        nc.tensor.matmul(ap[:], kv_perm[:, ts(h, HD)], qs[:],
                         start=False, stop=True)
        asb = b2(f"asb{h}", width=t_core)
        nc.vector.tensor_copy(asb[:], ap[:])
        attn_tiles.append(asb)

    # ---- Output projection: y[t, dout] = attn @ Wo^T, token-major ----
    for pair in range(CHUNKS // PAIR):
        psums = [psum(f"op{i}", N_CHUNK) for i in range(T_TILES * PAIR)]
        for dmid in range(DIN):
            wt = b4(f"wo{dmid}")
            nc.scalar.dma_start(wt[:], io["woT"][ts(dmid, P),
                                                 ts(pair, PAIR * N_CHUNK)])
            for t in range(T_TILES):
                for p in range(PAIR):
                    nc.tensor.matmul(
                        psums[t * PAIR + p][:],
                        attn_tiles[dmid][:, ts(t, P)],
                        wt[:, ts(p, N_CHUNK)],
                        start=(dmid == 0),
                        stop=(dmid == DIN - 1),
                    )
        for t in range(T_TILES):
            for p in range(PAIR):
                chunk = pair * PAIR + p
                osb = b2(f"osb{chunk}_{t}", dtype=F32)
                nc.vector.tensor_copy(osb[:], psums[t * PAIR + p][:])
                nc.gpsimd.dma_start(io["y"][ts(t, P), ts(chunk, N_CHUNK)],
                                    osb[:])


def build_nc(t_core, d_model, num_devices, replica_groups, reps=1):
    nc = bacc.Bacc("TRN2", target_bir_lowering=False, debug=False,
                   num_devices=num_devices)
    io = {}
    io["xT"] = nc.dram_tensor("xT", [d_model, t_core], F32R,
                              kind="ExternalInput").ap()
    for nm in ("wqT", "wkT", "wvT", "woT"):
        io[nm] = nc.dram_tensor(nm, [d_model, d_model], F32R,
                                kind="ExternalInput").ap()
    for nm in ("bkb", "bvb"):
        io[nm] = nc.dram_tensor(nm, [P, d_model], F32R,
                                kind="ExternalInput").ap()
    io["bqd"] = nc.dram_tensor("bqd", [P, d_model // P], F32,
                               kind="ExternalInput").ap()
    for nm in ("cosk", "sinkf"):
        io[nm] = nc.dram_tensor(nm, [t_core, HD], F32R,
                                kind="ExternalInput").ap()
    for nm in ("cosqD", "sinqD"):
        io[nm] = nc.dram_tensor(nm, [P, t_core], F32R,
                                kind="ExternalInput").ap()
    io["y"] = nc.dram_tensor("y", [t_core, d_model], F32,
                             kind="ExternalOutput").ap()

    with tile.TileContext(nc) as tc:
        for _ in range(reps):
            with ExitStack() as ctx:
                emit_attn(tc, ctx, io, t_core, d_model, replica_groups)
    nc.compile()
    return nc


# ---------------- host side ----------------

B, S, D = 2, 2048, 2048
NH_FULL = 16
MAX_POS = 4096
ROPE_THETA = 10000.0
N_CORES = 8
T_CORE = B * S // N_CORES

_cache = {}


def _rope_tables():
    inv_freq = (np.float32(1.0) /
                np.power(np.float32(ROPE_THETA),
                         np.arange(0, HD, 2, dtype=np.float32) /
                         np.float32(HD))).astype(np.float32)
    t = np.arange(MAX_POS, dtype=np.float32)
    freqs = np.outer(t, inv_freq).astype(np.float32)
    emb = np.concatenate((freqs, freqs), axis=-1)
    return np.cos(emb).astype(np.float32), np.sin(emb).astype(np.float32)


def _get_nc():
    if "nc" not in _cache:
        _cache["nc"] = build_nc(T_CORE, D, N_CORES,
                                [[0, 1, 2, 3], [4, 5, 6, 7]])
    return _cache["nc"]


def _host_inputs(hidden_states, position_ids, Wq, bq, Wk, bk, Wv, bv, Wo):
    x = np.asarray(hidden_states, dtype=np.float32).reshape(B * S, D)
    pos = np.asarray(position_ids).astype(np.int64).reshape(B * S)

    cos_t, sin_t = _rope_tables()
    cos = cos_t[pos]            # [B*S, HD]
    sin = sin_t[pos]
    # token-major k tables: sign-folded sin + 1/sqrt(HD) fold
    sinf = sin.copy()
    sinf[:, : HD // 2] *= np.float32(-1.0)
    scale = np.float32(1.0 / math.sqrt(HD))
    # feature-major q tables: sin* = +sin (i<64), -sin (i>=64)
    sinq = sin.copy()
    sinq[:, HD // 2:] *= np.float32(-1.0)

    wqT = np.ascontiguousarray(np.asarray(Wq, np.float32).T)
    wkT = np.ascontiguousarray(np.asarray(Wk, np.float32).T)
    wvT = np.ascontiguousarray(np.asarray(Wv, np.float32).T)
    woT = np.ascontiguousarray(np.asarray(Wo, np.float32).T)
    bkb = np.ascontiguousarray(np.broadcast_to(np.asarray(bk, np.float32), (P, D)))
    bvb = np.ascontiguousarray(np.broadcast_to(np.asarray(bv, np.float32), (P, D)))
    bqd = np.ascontiguousarray(np.asarray(bq, np.float32).reshape(D // P, P).T)

    in_maps = []
    for c in range(N_CORES):
        sl = slice(c * T_CORE, (c + 1) * T_CORE)
        in_maps.append({
            "xT": np.ascontiguousarray(x[sl].T),
            "wqT": wqT, "wkT": wkT, "wvT": wvT, "woT": woT,
            "bkb": bkb, "bvb": bvb, "bqd": bqd,
            "cosk": np.ascontiguousarray(cos[sl] * scale),
            "sinkf": np.ascontiguousarray(sinf[sl] * scale),
            "cosqD": np.ascontiguousarray(cos[sl].T),
            "sinqD": np.ascontiguousarray(sinq[sl].T),
        })
    return in_maps


def kernel(hidden_states, position_ids, Wq, bq, Wk, bk, Wv, bv, Wo):
    in_maps = _host_inputs(hidden_states, position_ids,
                           Wq, bq, Wk, bk, Wv, bv, Wo)
    nc = _get_nc()
    last_err = None
    for attempt in range(3):
        try:
            res = bass_utils.run_bass_kernel_spmd(
                nc, in_maps, core_ids=list(range(N_CORES)))
            break
        except Exception as e:  # transient axon/device states clear on retry
            last_err = e
            import time
            time.sleep(15 * (attempt + 1))
    else:
        raise last_err
    out = np.concatenate([res.results[c]["y"] for c in range(N_CORES)], axis=0)
    return out.reshape(B, S, D)


# revision 29
# speedup vs baseline: 1.0854x; 1.0300x over previous
"""Bass/Trainium2 kernel for nn_DreamAttention (dense transformer attention,
dead-softmax variant).

Math (per reference): q/k/v linear projections + RoPE, scores = q @ k^T /
sqrt(HD) (softmax computed but DISCARDED in the source), out = (scores @ v)
@ Wo^T.

Because no softmax is applied, attention is linear:
    (q @ k^T) @ v == q @ (k^T @ v)
so we compute the tiny per-head Gram matrix KV = k^T v  [HD, HD] instead of
the S x S score matrix (16x fewer FLOPs, no S x S materialization).

The q-side RoPE is folded into the attention matmul (RoPE is linear):
    attn_h = KV_h^T (cos*q_h) + KVp_h^T (sin* * q_h)
where KVp is KV with its partition halves swapped and sin* carries the
rotate-half signs. This lets the q projection emit feature-major tiles
directly (weight-stationary matmul), avoiding a transpose stage.

Sharding: data-parallel over tokens. 8 cores x 512 tokens (cores 0-3 hold
batch 0, cores 4-7 batch 1). Each core computes q/k/v for its tokens
(weights replicated), partial per-head KV over its tokens, an AllReduce of
the 1MB KV block within each 4-core batch group (overlapped with the q
projection), then attn and the output projection for its tokens. The scale
1/sqrt(HD) is folded into k's RoPE tables on the host.

Matmuls run in float32r (fp32 data, single-pass PE mode: full rate at free
dim >= 256 vs 4x slower true fp32). DMA triggers are spread across the SP
and ACT sequencers to keep trigger issue off the critical path.
"""

import math
from contextlib import ExitStack

import numpy as np

import concourse.mybir as mybir
import concourse.tile as tile
from concourse import bacc
from concourse import bass_utils

P = 128
HD = 128
F32 = mybir.dt.float32
F32R = mybir.dt.float32r


def ts(i, size):
    return slice(i * size, (i + 1) * size)


def emit_attn(tc, ctx, io, t_core, d_model, replica_groups):
    """Emit the per-core attention kernel.

    io: DRAM APs: xT [d_model, t_core]; wqT/wkT/wvT/woT [d_model, d_model];
    bkb/bvb [128, d_model] (broadcast biases); bqd [128, d_model/128]
    (bq in feature-major per-tile columns); cosk/sinkf [t_core, d_model]
    (token-major k tables, sign-folded + 1/sqrt(HD) prescaled);
    cosqD/sinqD [128, t_core] (feature-major q tables, sinqD sign-folded);
    y [t_core, d_model].
    """
    nc = tc.nc
    T_TILES = t_core // P
    DIN = d_model // P          # number of 128-wide feature tiles
    NH = d_model // HD          # heads
    N_CHUNK = min(512, d_model)
    CHUNKS = d_model // N_CHUNK
    PAIR = 2 if CHUNKS % 2 == 0 else 1   # chunks per W load / psum round
    QG = min(8, DIN)            # q-proj feature tiles per psum round

    sb = ctx.enter_context(tc.tile_pool(name="sb", bufs=1))
    ps = ctx.enter_context(tc.tile_pool(name="ps", bufs=8, space="PSUM"))
    dram = ctx.enter_context(tc.tile_pool(name="dram", bufs=4, space="DRAM"))

    def b2(name, dtype=F32R, width=None):
        return sb.tile([P, width or min(512, t_core)], dtype, name=name,
                       tag="b2", bufs=38)

    def b4(name, dtype=F32R):
        return sb.tile([P, PAIR * N_CHUNK], dtype, name=name, tag="b4", bufs=6)

    def b8(name, dtype=F32R):
        return sb.tile([P, d_model], dtype, name=name, tag="b8", bufs=11)

    def psum(name, width, dtype=F32):
        return ps.tile([P, width], dtype, name=name, tag="ps", bufs=8)

    # Resident x^T tiles [din, t] (matmul operand for all three projections)
    xt_tiles = []
    for din in range(DIN):
        xt = sb.tile([P, t_core], F32R, name=f"xt{din}", tag="b2", bufs=38)
        nc.scalar.dma_start(xt[:], io["xT"][ts(din, P), :])
        xt_tiles.append(xt)

    def project_tmajor(wT_ap, bias_ap, out_tiles, dma_eng, pairs=None):
        """out[t, dout] = x @ W^T + b, token-major tiles [128, d_model]."""
        for pair in (range(CHUNKS // PAIR) if pairs is None else pairs):
            psums = [psum(f"pp{i}", N_CHUNK) for i in range(T_TILES * PAIR)]
            for din in range(DIN):
                wt = b4(f"w{din}")
                dma_eng.dma_start(wt[:], wT_ap[ts(din, P),
                                               ts(pair, PAIR * N_CHUNK)])
                for t in range(T_TILES):
                    for p in range(PAIR):
                        nc.tensor.matmul(
                            psums[t * PAIR + p][:],
                            xt_tiles[din][:, ts(t, P)],
                            wt[:, ts(p, N_CHUNK)],
                            start=(din == 0),
                            stop=(din == DIN - 1),
                        )
            bt = b4(f"bias_p{pair}")
            dma_eng.dma_start(bt[:], bias_ap[:, ts(pair, PAIR * N_CHUNK)])
            for t in range(T_TILES):
                for p in range(PAIR):
                    chunk = pair * PAIR + p
                    nc.vector.tensor_add(
                        out_tiles[t][:, ts(chunk, N_CHUNK)],
                        psums[t * PAIR + p][:],
                        bt[:, ts(p, N_CHUNK)],
                    )

    def rope_tmajor(tiles, cos_ap, sinf_ap, t_range=None):
        """In-place RoPE on token-major tiles using compact [t, HD] tables
        broadcast across heads.

        out = x*cos + rot_half(x)*sin; sinf is sign-folded so
        rot_half(x)*sin == gather(x, +-64) * sinf elementwise.
        """
        h2 = HD // 2

        def bc(ap2d):  # [128, w] -> [128, NH, w] broadcast view
            return ap2d.unsqueeze(1).broadcast_to([P, NH, ap2d.shape[-1]])

        for t in (range(T_TILES) if t_range is None else t_range):
            ct = b2(f"cos{t}", width=HD)
            st = b2(f"sin{t}", width=HD)
            nc.scalar.dma_start(ct[:], cos_ap[ts(t, P), :])
            nc.scalar.dma_start(st[:], sinf_ap[ts(t, P), :])
            tmp = b8(f"ropetmp{t}")
            x3 = tiles[t][:].rearrange("p (h d) -> p h d", d=HD)
            t3 = tmp[:].rearrange("p (h d) -> p h d", d=HD)
            nc.vector.tensor_mul(t3[:, :, 0:h2], x3[:, :, h2:HD],
                                 bc(st[:, 0:h2]))
            nc.vector.tensor_mul(t3[:, :, h2:HD], x3[:, :, 0:h2],
                                 bc(st[:, h2:HD]))
            nc.vector.tensor_mul(x3, x3, bc(ct[:]))
            nc.vector.tensor_add(tiles[t][:], tiles[t][:], tmp[:])

    # ---- K/V projections + RoPE(k) ----
    k_tiles = [b8(f"k{t}") for t in range(T_TILES)]
    project_tmajor(io["wkT"], io["bkb"], k_tiles, nc.sync)
    rope_tmajor(k_tiles, io["cosk"], io["sinkf"])
    v_tiles = [b8(f"v{t}") for t in range(T_TILES)]
    project_tmajor(io["wvT"], io["bvb"], v_tiles, nc.scalar)

    # ---- Per-head Gram matrices KV[h] = k_h^T @ v_h (partial over this
    # core's tokens), packed [128, NH*HD]. The moving operand spans two
    # heads (N=256) to stay at full fp32r rate; the unwanted half of each
    # product is discarded at eviction. ----
    kv_sb = b8("kvsb")

    def kv_partials(h_lo, h_hi):
        for h in range(h_lo, h_hi):
            base = min(h, NH - 2) if NH >= 2 else 0
            good = h - base
            width = min(2 * HD, (NH - base) * HD)
            kvp = psum(f"kvp{h}", width)
            for t in range(T_TILES):
                nc.tensor.matmul(
                    kvp[:],
                    k_tiles[t][:, ts(h, HD)],
                    v_tiles[t][:, base * HD: base * HD + width],
                    start=(t == 0),
                    stop=(t == T_TILES - 1),
                )
            nc.vector.tensor_copy(kv_sb[:, ts(h, HD)], kvp[:, ts(good, HD)])

    # ---- AllReduce the KV partials within the batch group, in two halves
    # so attention on the first heads can start while the second half is
    # still reducing ----
    kv_red = b8("kvred")
    kv_perm = b8("kvperm")
    h2 = HD // 2
    N_SPLIT = 2 if NH >= 2 else 1
    HS = NH // N_SPLIT            # heads per collective
    W_HALF = HS * HD
    for s in range(N_SPLIT):
        kv_partials(s * HS, (s + 1) * HS)
        kv_in = dram.tile([P, W_HALF], F32R, name=f"kv_in{s}")
        kv_out = dram.tile([P, W_HALF], F32R, name=f"kv_out{s}")
        nc.gpsimd.dma_start(kv_in[:], kv_sb[:, ts(s, W_HALF)])
        nc.gpsimd.collective_compute(
            "AllReduce",
            mybir.AluOpType.add,
            replica_groups=replica_groups,
            ins=[kv_in.opt()],
            outs=[kv_out.opt()],
        )
        nc.gpsimd.dma_start(kv_red[:, ts(s, W_HALF)], kv_out[:])
        # Partition-half-swapped copy for the folded q-side RoPE
        nc.gpsimd.dma_start(kv_perm[0:h2, ts(s, W_HALF)], kv_out[h2:HD, :])
        nc.gpsimd.dma_start(kv_perm[h2:HD, ts(s, W_HALF)], kv_out[0:h2, :])

    # ---- Q projection, feature-major: qD[dout, t] = W q-row blocks ----
    qd_tiles = [None] * DIN
    bqd_sb = b2("bqd", dtype=F32, width=DIN)
    nc.sync.dma_start(bqd_sb[:], io["bqd"][:])
    for g in range(DIN // QG):
        psums = [psum(f"qp{i}", t_core) for i in range(QG)]
        for din in range(DIN):
            wt = b4(f"wq{din}")
            nc.sync.dma_start(wt[:], io["wqT"][ts(din, P),
                                               ts(g, QG * P)])
            for j in range(QG):
                nc.tensor.matmul(
                    psums[j][:],
                    wt[:, ts(j, P)],
                    xt_tiles[din][:],
                    start=(din == 0),
                    stop=(din == DIN - 1),
                )
        for j in range(QG):
            dout = g * QG + j
            qd = b2(f"qd{dout}", width=t_core)
            nc.vector.tensor_scalar_add(qd[:], psums[j][:],
                                        bqd_sb[:, dout:dout + 1])
            qd_tiles[dout] = qd

    cosq = b2("cosq", width=t_core)
    sinq = b2("sinq", width=t_core)
    nc.scalar.dma_start(cosq[:], io["cosqD"][:])
    nc.scalar.dma_start(sinq[:], io["sinqD"][:])

    # ---- attn_h[d2, t] = KV_h^T (cos*q_h) + KVp_h^T (sin* q_h) ----
    attn_tiles = []
    for h in range(NH):
        qc = b2(f"qc{h}", width=t_core)
        nc.vector.tensor_mul(qc[:], qd_tiles[h][:], cosq[:])
        qs = b2(f"qs{h}", width=t_core)
        nc.vector.tensor_mul(qs[:], qd_tiles[h][:], sinq[:])
        ap = psum(f"ap{h}", t_core)
        nc.tensor.matmul(ap[:], kv_red[:, ts(h, HD)], qc[:],
                         start=True, stop=False)
        nc.tensor.matmul(ap[:], kv_perm[:, ts(h, HD)], qs[:],
                         start=False, stop=True)
        asb = b2(f"asb{h}", width=t_core)
        nc.vector.tensor_copy(asb[:], ap[:])
        attn_tiles.append(asb)

    # ---- Output projection: y[t, dout] = attn @ Wo^T, token-major ----
    for pair in range(CHUNKS // PAIR):
        psums = [psum(f"op{i}", N_CHUNK) for i in range(T_TILES * PAIR)]
        for dmid in range(DIN):
            wt = b4(f"wo{dmid}")
            nc.scalar.dma_start(wt[:], io["woT"][ts(dmid, P),
                                                 ts(pair, PAIR * N_CHUNK)])
            for t in range(T_TILES):
                for p in range(PAIR):
                    nc.tensor.matmul(
                        psums[t * PAIR + p][:],
                        attn_tiles[dmid][:, ts(t, P)],
                        wt[:, ts(p, N_CHUNK)],
                        start=(dmid == 0),
                        stop=(dmid == DIN - 1),
                    )
        for t in range(T_TILES):
            for p in range(PAIR):
                chunk = pair * PAIR + p
                osb = b2(f"osb{chunk}_{t}", dtype=F32)
                nc.vector.tensor_copy(osb[:], psums[t * PAIR + p][:])
                nc.gpsimd.dma_start(io["y"][ts(t, P), ts(chunk, N_CHUNK)],
                                    osb[:])


def build_nc(t_core, d_model, num_devices, replica_groups, reps=1):
    nc = bacc.Bacc("TRN2", target_bir_lowering=False, debug=False,
                   num_devices=num_devices)
    io = {}
    io["xT"] = nc.dram_tensor("xT", [d_model, t_core], F32R,
                              kind="ExternalInput").ap()
    for nm in ("wqT", "wkT", "wvT", "woT"):
        io[nm] = nc.dram_tensor(nm, [d_model, d_model], F32R,
                                kind="ExternalInput").ap()
    for nm in ("bkb", "bvb"):
        io[nm] = nc.dram_tensor(nm, [P, d_model], F32R,
                                kind="ExternalInput").ap()
    io["bqd"] = nc.dram_tensor("bqd", [P, d_model // P], F32,
                               kind="ExternalInput").ap()
    for nm in ("cosk", "sinkf"):
        io[nm] = nc.dram_tensor(nm, [t_core, HD], F32R,
                                kind="ExternalInput").ap()
    for nm in ("cosqD", "sinqD"):
        io[nm] = nc.dram_tensor(nm, [P, t_core], F32R,
                                kind="ExternalInput").ap()
    io["y"] = nc.dram_tensor("y", [t_core, d_model], F32,
                             kind="ExternalOutput").ap()

    with tile.TileContext(nc) as tc:
        for _ in range(reps):
            with ExitStack() as ctx:
                emit_attn(tc, ctx, io, t_core, d_model, replica_groups)
    nc.compile()
    return nc


# ---------------- host side ----------------

B, S, D = 2, 2048, 2048
NH_FULL = 16
MAX_POS = 4096
ROPE_THETA = 10000.0
N_CORES = 8
T_CORE = B * S // N_CORES

_cache = {}


def _rope_tables():
    inv_freq = (np.float32(1.0) /
                np.power(np.float32(ROPE_THETA),
                         np.arange(0, HD, 2, dtype=np.float32) /
                         np.float32(HD))).astype(np.float32)
    t = np.arange(MAX_POS, dtype=np.float32)
    freqs = np.outer(t, inv_freq).astype(np.float32)
    emb = np.concatenate((freqs, freqs), axis=-1)
    return np.cos(emb).astype(np.float32), np.sin(emb).astype(np.float32)


def _get_nc():
    if "nc" not in _cache:
        _cache["nc"] = build_nc(T_CORE, D, N_CORES,
                                [[0, 1, 2, 3], [4, 5, 6, 7]])
    return _cache["nc"]


def _host_inputs(hidden_states, position_ids, Wq, bq, Wk, bk, Wv, bv, Wo):
    x = np.asarray(hidden_states, dtype=np.float32).reshape(B * S, D)
    pos = np.asarray(position_ids).astype(np.int64).reshape(B * S)

    cos_t, sin_t = _rope_tables()
    cos = cos_t[pos]            # [B*S, HD]
    sin = sin_t[pos]
    # token-major k tables: sign-folded sin + 1/sqrt(HD) fold
    sinf = sin.copy()
    sinf[:, : HD // 2] *= np.float32(-1.0)
    scale = np.float32(1.0 / math.sqrt(HD))
    # feature-major q tables: sin* = +sin (i<64), -sin (i>=64)
    sinq = sin.copy()
    sinq[:, HD // 2:] *= np.float32(-1.0)

    wqT = np.ascontiguousarray(np.asarray(Wq, np.float32).T)
    wkT = np.ascontiguousarray(np.asarray(Wk, np.float32).T)
    wvT = np.ascontiguousarray(np.asarray(Wv, np.float32).T)
    woT = np.ascontiguousarray(np.asarray(Wo, np.float32).T)
    bkb = np.ascontiguousarray(np.broadcast_to(np.asarray(bk, np.float32), (P, D)))
    bvb = np.ascontiguousarray(np.broadcast_to(np.asarray(bv, np.float32), (P, D)))
    bqd = np.ascontiguousarray(np.asarray(bq, np.float32).reshape(D // P, P).T)

    in_maps = []
    for c in range(N_CORES):
        sl = slice(c * T_CORE, (c + 1) * T_CORE)
        in_maps.append({
            "xT": np.ascontiguousarray(x[sl].T),
            "wqT": wqT, "wkT": wkT, "wvT": wvT, "woT": woT,
            "bkb": bkb, "bvb": bvb, "bqd": bqd,
            "cosk": np.ascontiguousarray(cos[sl] * scale),
            "sinkf": np.ascontiguousarray(sinf[sl] * scale),
            "cosqD": np.ascontiguousarray(cos[sl].T),
            "sinqD": np.ascontiguousarray(sinq[sl].T),
        })
    return in_maps


def kernel(hidden_states, position_ids, Wq, bq, Wk, bk, Wv, bv, Wo):
    in_maps = _host_inputs(hidden_states, position_ids,
                           Wq, bq, Wk, bk, Wv, bv, Wo)
    nc = _get_nc()
    last_err = None
    for attempt in range(3):
        try:
            res = bass_utils.run_bass_kernel_spmd(
                nc, in_maps, core_ids=list(range(N_CORES)))
            break
        except Exception as e:  # transient axon/device states clear on retry
            last_err = e
            import time
            time.sleep(15 * (attempt + 1))
    else:
        raise last_err
    out = np.concatenate([res.results[c]["y"] for c in range(N_CORES)], axis=0)
    return out.reshape(B, S, D)


# revision 34
# speedup vs baseline: 1.4055x; 1.2949x over previous
"""Bass/Trainium2 kernel for nn_DreamAttention (dense transformer attention,
dead-softmax variant).

Math (per reference): q/k/v linear projections + RoPE, scores = q @ k^T /
sqrt(HD) (softmax computed but DISCARDED in the source), out = (scores @ v)
@ Wo^T.

Because no softmax is applied, attention is linear:
    (q @ k^T) @ v == q @ (k^T @ v)
so we compute the tiny per-head Gram matrix KV = k^T v  [HD, HD] instead of
the S x S score matrix (16x fewer FLOPs, no S x S materialization).

The q-side RoPE is folded into the attention matmul (RoPE is linear):
    attn_h = KV_h^T (cos*q_h) + KVp_h^T (sin* * q_h)
where KVp is KV with its partition halves swapped and sin* carries the
rotate-half signs. This lets the q projection emit feature-major tiles
directly (weight-stationary matmul), avoiding a transpose stage.

Sharding: data-parallel over tokens. 8 cores x 512 tokens (cores 0-3 hold
batch 0, cores 4-7 batch 1). Each core computes q/k/v for its tokens
(weights replicated), partial per-head KV over its tokens, an AllReduce of
the 1MB KV block within each 4-core batch group (overlapped with the q
projection), then attn and the output projection for its tokens. The scale
1/sqrt(HD) is folded into k's RoPE tables on the host.

Matmuls run in float32r (fp32 data, single-pass PE mode: full rate at free
dim >= 256 vs 4x slower true fp32). DMA triggers are spread across the SP
and ACT sequencers to keep trigger issue off the critical path.
"""

import math
from contextlib import ExitStack

import numpy as np

import concourse.mybir as mybir
import concourse.tile as tile
from concourse import bacc
from concourse import bass_utils

P = 128
HD = 128
F32 = mybir.dt.float32
F32R = mybir.dt.float32r


def ts(i, size):
    return slice(i * size, (i + 1) * size)


def emit_attn(tc, ctx, io, t_core, d_model, replica_groups):
    """Emit the per-core attention kernel.

    io: DRAM APs: xT [d_model, t_core]; wqT/wkT/wvT/woT [d_model, d_model];
    bkb/bvb [128, d_model] (broadcast biases); bqd [128, d_model/128]
    (bq in feature-major per-tile columns); cosk/sinkf [t_core, d_model]
    (token-major k tables, sign-folded + 1/sqrt(HD) prescaled);
    cosqD/sinqD [128, t_core] (feature-major q tables, sinqD sign-folded);
    y [t_core, d_model].
    """
    nc = tc.nc
    T_TILES = t_core // P
    DIN = d_model // P          # number of 128-wide feature tiles
    NH = d_model // HD          # heads
    N_CHUNK = min(512, d_model)
    CHUNKS = d_model // N_CHUNK
    PAIR = 2 if CHUNKS % 2 == 0 else 1   # chunks per W load / psum round
    QG = min(8, DIN)            # q-proj feature tiles per psum round

    sb = ctx.enter_context(tc.tile_pool(name="sb", bufs=1))
    ps = ctx.enter_context(tc.tile_pool(name="ps", bufs=8, space="PSUM"))
    dram = ctx.enter_context(tc.tile_pool(name="dram", bufs=4, space="DRAM"))

    def b2(name, dtype=F32R, width=None):
        return sb.tile([P, width or min(512, t_core)], dtype, name=name,
                       tag="b2", bufs=38)

    def b4(name, dtype=F32R, width=None):
        w = width or PAIR * N_CHUNK
        if w <= min(512, t_core):
            return b2(name, dtype, width=w)
        return sb.tile([P, w], dtype, name=name, tag="b4", bufs=6)

    def b8(name, dtype=F32R):
        return sb.tile([P, d_model], dtype, name=name, tag="b8", bufs=11)

    def psum(name, width, dtype=F32):
        return ps.tile([P, width], dtype, name=name, tag="ps", bufs=8)

    # Resident x^T tiles [din, t] (matmul operand for all three projections)
    xt_tiles = []
    for din in range(DIN):
        xt = sb.tile([P, t_core], F32R, name=f"xt{din}", tag="b2", bufs=38)
        nc.scalar.dma_start(xt[:], io["xT"][ts(din, P), :])
        xt_tiles.append(xt)

    def project_tmajor(wT_ap, bias_ap, out_tiles, dma_eng, pairs=None):
        """out[t, dout] = x @ W^T + b, token-major tiles [128, d_model]."""
        for pair in (range(CHUNKS // PAIR) if pairs is None else pairs):
            psums = [psum(f"pp{i}", N_CHUNK) for i in range(T_TILES * PAIR)]
            for din in range(DIN):
                wt = b4(f"w{din}")
                dma_eng.dma_start(wt[:], wT_ap[ts(din, P),
                                               ts(pair, PAIR * N_CHUNK)])
                for t in range(T_TILES):
                    for p in range(PAIR):
                        nc.tensor.matmul(
                            psums[t * PAIR + p][:],
                            xt_tiles[din][:, ts(t, P)],
                            wt[:, ts(p, N_CHUNK)],
                            start=(din == 0),
                            stop=(din == DIN - 1),
                        )
            bt = b4(f"bias_p{pair}")
            dma_eng.dma_start(bt[:], bias_ap[:, ts(pair, PAIR * N_CHUNK)])
            for t in range(T_TILES):
                for p in range(PAIR):
                    chunk = pair * PAIR + p
                    nc.vector.tensor_add(
                        out_tiles[t][:, ts(chunk, N_CHUNK)],
                        psums[t * PAIR + p][:],
                        bt[:, ts(p, N_CHUNK)],
                    )

    def rope_tmajor(tiles, cos_ap, sinf_ap, t_range=None):
        """In-place RoPE on token-major tiles using compact [t, HD] tables
        broadcast across heads.

        out = x*cos + rot_half(x)*sin; sinf is sign-folded so
        rot_half(x)*sin == gather(x, +-64) * sinf elementwise.
        """
        h2 = HD // 2

        def bc(ap2d):  # [128, w] -> [128, NH, w] broadcast view
            return ap2d.unsqueeze(1).broadcast_to([P, NH, ap2d.shape[-1]])

        for t in (range(T_TILES) if t_range is None else t_range):
            ct = b2(f"cos{t}", width=HD)
            st = b2(f"sin{t}", width=HD)
            nc.scalar.dma_start(ct[:], cos_ap[ts(t, P), :])
            nc.scalar.dma_start(st[:], sinf_ap[ts(t, P), :])
            tmp = b8(f"ropetmp{t}")
            x3 = tiles[t][:].rearrange("p (h d) -> p h d", d=HD)
            t3 = tmp[:].rearrange("p (h d) -> p h d", d=HD)
            nc.vector.tensor_mul(t3[:, :, 0:h2], x3[:, :, h2:HD],
                                 bc(st[:, 0:h2]))
            nc.vector.tensor_mul(t3[:, :, h2:HD], x3[:, :, 0:h2],
                                 bc(st[:, h2:HD]))
            nc.vector.tensor_mul(x3, x3, bc(ct[:]))
            nc.vector.tensor_add(tiles[t][:], tiles[t][:], tmp[:])

    # ---- K/V projections + RoPE(k) ----
    k_tiles = [b8(f"k{t}") for t in range(T_TILES)]
    project_tmajor(io["wkT"], io["bkb"], k_tiles, nc.sync)
    rope_tmajor(k_tiles, io["cosk"], io["sinkf"])
    v_tiles = [b8(f"v{t}") for t in range(T_TILES)]
    project_tmajor(io["wvT"], io["bvb"], v_tiles, nc.scalar)

    # ---- Per-head Gram matrices KV[h] = k_h^T @ v_h (partial over this
    # core's tokens), packed [128, NH*HD]. The moving operand spans two
    # heads (N=256) to stay at full fp32r rate; the unwanted half of each
    # product is discarded at eviction. ----
    kv_sb = b8("kvsb")

    def kv_partials(h_lo, h_hi):
        for h in range(h_lo, h_hi):
            base = min(h, NH - 2) if NH >= 2 else 0
            good = h - base
            width = min(2 * HD, (NH - base) * HD)
            kvp = psum(f"kvp{h}", width)
            for t in range(T_TILES):
                nc.tensor.matmul(
                    kvp[:],
                    k_tiles[t][:, ts(h, HD)],
                    v_tiles[t][:, base * HD: base * HD + width],
                    start=(t == 0),
                    stop=(t == T_TILES - 1),
                )
            nc.vector.tensor_copy(kv_sb[:, ts(h, HD)], kvp[:, ts(good, HD)])

    # ---- AllReduce the KV partials within the batch group, in two halves
    # so attention on the first heads can start while the second half is
    # still reducing ----
    kv_red = b8("kvred")
    kv_perm = b8("kvperm")
    h2 = HD // 2
    N_SPLIT = 2 if NH >= 2 else 1
    HS = NH // N_SPLIT            # heads per collective
    W_HALF = HS * HD
    for s in range(N_SPLIT):
        kv_partials(s * HS, (s + 1) * HS)
        kv_in = dram.tile([P, W_HALF], F32R, name=f"kv_in{s}")
        kv_out = dram.tile([P, W_HALF], F32R, name=f"kv_out{s}")
        nc.gpsimd.dma_start(kv_in[:], kv_sb[:, ts(s, W_HALF)])
        nc.gpsimd.collective_compute(
            "AllReduce",
            mybir.AluOpType.add,
            replica_groups=replica_groups,
            ins=[kv_in.opt()],
            outs=[kv_out.opt()],
        )
        nc.gpsimd.dma_start(kv_red[:, ts(s, W_HALF)], kv_out[:])
        # Partition-half-swapped copy for the folded q-side RoPE
        nc.gpsimd.dma_start(kv_perm[0:h2, ts(s, W_HALF)], kv_out[h2:HD, :])
        nc.gpsimd.dma_start(kv_perm[h2:HD, ts(s, W_HALF)], kv_out[0:h2, :])

    # ---- Q projection, feature-major: qD[dout, t] = W q-row blocks ----
    qd_tiles = [None] * DIN
    bqd_sb = b2("bqd", dtype=F32, width=DIN)
    nc.sync.dma_start(bqd_sb[:], io["bqd"][:])
    for g in range(DIN // QG):
        psums = [psum(f"qp{i}", t_core) for i in range(QG)]
        for din in range(DIN):
            wt = b4(f"wq{din}", width=QG * P)
            nc.sync.dma_start(wt[:], io["wqT"][ts(din, P),
                                               ts(g, QG * P)])
            for j in range(QG):
                nc.tensor.matmul(
                    psums[j][:],
                    wt[:, ts(j, P)],
                    xt_tiles[din][:],
                    start=(din == 0),
                    stop=(din == DIN - 1),
                )
        for j in range(QG):
            dout = g * QG + j
            qd = b2(f"qd{dout}", width=t_core)
            nc.vector.tensor_scalar_add(qd[:], psums[j][:],
                                        bqd_sb[:, dout:dout + 1])
            qd_tiles[dout] = qd

    cosq = b2("cosq", width=t_core)
    sinq = b2("sinq", width=t_core)
    nc.scalar.dma_start(cosq[:], io["cosqD"][:])
    nc.scalar.dma_start(sinq[:], io["sinqD"][:])

    # ---- attn_h[d2, t] = KV_h^T (cos*q_h) + KVp_h^T (sin* q_h) ----
    attn_tiles = []
    for h in range(NH):
        qc = b2(f"qc{h}", width=t_core)
        nc.vector.tensor_mul(qc[:], qd_tiles[h][:], cosq[:])
        qs = b2(f"qs{h}", width=t_core)
        nc.vector.tensor_mul(qs[:], qd_tiles[h][:], sinq[:])
        ap = psum(f"ap{h}", t_core)
        nc.tensor.matmul(ap[:], kv_red[:, ts(h, HD)], qc[:],
                         start=True, stop=False)
        nc.tensor.matmul(ap[:], kv_perm[:, ts(h, HD)], qs[:],
                         start=False, stop=True)
        asb = b2(f"asb{h}", width=t_core)
        nc.vector.tensor_copy(asb[:], ap[:])
        attn_tiles.append(asb)

    # ---- Output projection: y[t, dout] = attn @ Wo^T, token-major ----
    for pair in range(CHUNKS // PAIR):
        psums = [psum(f"op{i}", N_CHUNK) for i in range(T_TILES * PAIR)]
        for dmid in range(DIN):
            wt = b4(f"wo{dmid}")
            nc.scalar.dma_start(wt[:], io["woT"][ts(dmid, P),
                                                 ts(pair, PAIR * N_CHUNK)])
            for t in range(T_TILES):
                for p in range(PAIR):
                    nc.tensor.matmul(
                        psums[t * PAIR + p][:],
                        attn_tiles[dmid][:, ts(t, P)],
                        wt[:, ts(p, N_CHUNK)],
                        start=(dmid == 0),
                        stop=(dmid == DIN - 1),
                    )
        for t in range(T_TILES):
            for p in range(PAIR):
                chunk = pair * PAIR + p
                osb = b2(f"osb{chunk}_{t}", dtype=F32)
                nc.vector.tensor_copy(osb[:], psums[t * PAIR + p][:])
                nc.gpsimd.dma_start(io["y"][ts(t, P), ts(chunk, N_CHUNK)],
                                    osb[:])


def build_nc(t_core, d_model, num_devices, replica_groups, reps=1):
    nc = bacc.Bacc("TRN2", target_bir_lowering=False, debug=False,
                   num_devices=num_devices)
    io = {}
    io["xT"] = nc.dram_tensor("xT", [d_model, t_core], F32R,
                              kind="ExternalInput").ap()
    for nm in ("wqT", "wkT", "wvT", "woT"):
        io[nm] = nc.dram_tensor(nm, [d_model, d_model], F32R,
                                kind="ExternalInput").ap()
    for nm in ("bkb", "bvb"):
        io[nm] = nc.dram_tensor(nm, [P, d_model], F32R,
                                kind="ExternalInput").ap()
    io["bqd"] = nc.dram_tensor("bqd", [P, d_model // P], F32,
                               kind="ExternalInput").ap()
    for nm in ("cosk", "sinkf"):
        io[nm] = nc.dram_tensor(nm, [t_core, HD], F32R,
                                kind="ExternalInput").ap()
    for nm in ("cosqD", "sinqD"):
        io[nm] = nc.dram_tensor(nm, [P, t_core], F32R,
                                kind="ExternalInput").ap()
    io["y"] = nc.dram_tensor("y", [t_core, d_model], F32,
                             kind="ExternalOutput").ap()

    with tile.TileContext(nc) as tc:
        for _ in range(reps):
            with ExitStack() as ctx:
                emit_attn(tc, ctx, io, t_core, d_model, replica_groups)
    nc.compile()
    return nc


# ---------------- host side ----------------

B, S, D = 2, 2048, 2048
NH_FULL = 16
MAX_POS = 4096
ROPE_THETA = 10000.0
N_CORES = 8
T_CORE = B * S // N_CORES

_cache = {}


def _rope_tables():
    inv_freq = (np.float32(1.0) /
                np.power(np.float32(ROPE_THETA),
                         np.arange(0, HD, 2, dtype=np.float32) /
                         np.float32(HD))).astype(np.float32)
    t = np.arange(MAX_POS, dtype=np.float32)
    freqs = np.outer(t, inv_freq).astype(np.float32)
    emb = np.concatenate((freqs, freqs), axis=-1)
    return np.cos(emb).astype(np.float32), np.sin(emb).astype(np.float32)


def _get_nc():
    if "nc" not in _cache:
        _cache["nc"] = build_nc(T_CORE, D, N_CORES,
                                [[0, 1, 2, 3], [4, 5, 6, 7]])
    return _cache["nc"]


def _host_inputs(hidden_states, position_ids, Wq, bq, Wk, bk, Wv, bv, Wo):
    x = np.asarray(hidden_states, dtype=np.float32).reshape(B * S, D)
    pos = np.asarray(position_ids).astype(np.int64).reshape(B * S)

    cos_t, sin_t = _rope_tables()
    cos = cos_t[pos]            # [B*S, HD]
    sin = sin_t[pos]
    # token-major k tables: sign-folded sin + 1/sqrt(HD) fold
    sinf = sin.copy()
    sinf[:, : HD // 2] *= np.float32(-1.0)
    scale = np.float32(1.0 / math.sqrt(HD))
    # feature-major q tables: sin* = +sin (i<64), -sin (i>=64)
    sinq = sin.copy()
    sinq[:, HD // 2:] *= np.float32(-1.0)

    wqT = np.ascontiguousarray(np.asarray(Wq, np.float32).T)
    wkT = np.ascontiguousarray(np.asarray(Wk, np.float32).T)
    wvT = np.ascontiguousarray(np.asarray(Wv, np.float32).T)
    woT = np.ascontiguousarray(np.asarray(Wo, np.float32).T)
    bkb = np.ascontiguousarray(np.broadcast_to(np.asarray(bk, np.float32), (P, D)))
    bvb = np.ascontiguousarray(np.broadcast_to(np.asarray(bv, np.float32), (P, D)))
    bqd = np.ascontiguousarray(np.asarray(bq, np.float32).reshape(D // P, P).T)

    in_maps = []
    for c in range(N_CORES):
        sl = slice(c * T_CORE, (c + 1) * T_CORE)
        in_maps.append({
            "xT": np.ascontiguousarray(x[sl].T),
            "wqT": wqT, "wkT": wkT, "wvT": wvT, "woT": woT,
            "bkb": bkb, "bvb": bvb, "bqd": bqd,
            "cosk": np.ascontiguousarray(cos[sl] * scale),
            "sinkf": np.ascontiguousarray(sinf[sl] * scale),
            "cosqD": np.ascontiguousarray(cos[sl].T),
            "sinqD": np.ascontiguousarray(sinq[sl].T),
        })
    return in_maps


def kernel(hidden_states, position_ids, Wq, bq, Wk, bk, Wv, bv, Wo):
    in_maps = _host_inputs(hidden_states, position_ids,
                           Wq, bq, Wk, bk, Wv, bv, Wo)
    nc = _get_nc()
    last_err = None
    for attempt in range(3):
        try:
            res = bass_utils.run_bass_kernel_spmd(
                nc, in_maps, core_ids=list(range(N_CORES)))
            break
        except Exception as e:  # transient axon/device states clear on retry
            last_err = e
            import time
            time.sleep(15 * (attempt + 1))
    else:
        raise last_err
    out = np.concatenate([res.results[c]["y"] for c in range(N_CORES)], axis=0)
    return out.reshape(B, S, D)


# revision 35
# speedup vs baseline: 1.4984x; 1.0661x over previous
"""Bass/Trainium2 kernel for nn_DreamAttention (dense transformer attention,
dead-softmax variant).

Math (per reference): q/k/v linear projections + RoPE, scores = q @ k^T /
sqrt(HD) (softmax computed but DISCARDED in the source), out = (scores @ v)
@ Wo^T.

Because no softmax is applied, attention is linear:
    (q @ k^T) @ v == q @ (k^T @ v)
so we compute the tiny per-head Gram matrix KV = k^T v  [HD, HD] instead of
the S x S score matrix (16x fewer FLOPs, no S x S materialization).

The q-side RoPE is folded into the attention matmul (RoPE is linear):
    attn_h = KV_h^T (cos*q_h) + KVp_h^T (sin* * q_h)
where KVp is KV with its partition halves swapped and sin* carries the
rotate-half signs. This lets the q projection emit feature-major tiles
directly (weight-stationary matmul), avoiding a transpose stage.

Sharding: data-parallel over tokens. 8 cores x 512 tokens (cores 0-3 hold
batch 0, cores 4-7 batch 1). Each core computes q/k/v for its tokens
(weights replicated), partial per-head KV over its tokens, an AllReduce of
the 1MB KV block within each 4-core batch group (overlapped with the q
projection), then attn and the output projection for its tokens. The scale
1/sqrt(HD) is folded into k's RoPE tables on the host.

Matmuls run in float32r (fp32 data, single-pass PE mode: full rate at free
dim >= 256 vs 4x slower true fp32). DMA triggers are spread across the SP
and ACT sequencers to keep trigger issue off the critical path.
"""

import math
from contextlib import ExitStack

import numpy as np

import concourse.mybir as mybir
import concourse.tile as tile
from concourse import bacc
from concourse import bass_utils

P = 128
HD = 128
F32 = mybir.dt.float32
F32R = mybir.dt.float32r


def ts(i, size):
    return slice(i * size, (i + 1) * size)


def emit_attn(tc, ctx, io, t_core, d_model, replica_groups):
    """Emit the per-core attention kernel.

    io: DRAM APs: xT [d_model, t_core]; wqT/wkT/wvT/woT [d_model, d_model];
    bkb/bvb [128, d_model] (broadcast biases); bqd [128, d_model/128]
    (bq in feature-major per-tile columns); cosk/sinkf [t_core, d_model]
    (token-major k tables, sign-folded + 1/sqrt(HD) prescaled);
    cosqD/sinqD [128, t_core] (feature-major q tables, sinqD sign-folded);
    y [t_core, d_model].
    """
    nc = tc.nc
    T_TILES = t_core // P
    DIN = d_model // P          # number of 128-wide feature tiles
    NH = d_model // HD          # heads
    N_CHUNK = min(512, d_model)
    CHUNKS = d_model // N_CHUNK
    PAIR = 2 if CHUNKS % 2 == 0 else 1   # chunks per W load / psum round
    QG = min(8, DIN)            # q-proj feature tiles per psum round

    sb = ctx.enter_context(tc.tile_pool(name="sb", bufs=1))
    ps = ctx.enter_context(tc.tile_pool(name="ps", bufs=8, space="PSUM"))
    dram = ctx.enter_context(tc.tile_pool(name="dram", bufs=4, space="DRAM"))

    def b2(name, dtype=F32R, width=None):
        return sb.tile([P, width or min(512, t_core)], dtype, name=name,
                       tag="b2", bufs=38)

    def b4(name, dtype=F32R, width=None):
        w = width or PAIR * N_CHUNK
        if w <= min(512, t_core):
            return b2(name, dtype, width=w)
        return sb.tile([P, w], dtype, name=name, tag="b4", bufs=6)

    def b8(name, dtype=F32R):
        return sb.tile([P, d_model], dtype, name=name, tag="b8", bufs=11)

    def psum(name, width, dtype=F32):
        return ps.tile([P, width], dtype, name=name, tag="ps", bufs=8)

    # Resident x^T tiles [din, t] (matmul operand for all three projections)
    xt_tiles = []
    for din in range(DIN):
        xt = sb.tile([P, t_core], F32R, name=f"xt{din}", tag="b2", bufs=38)
        nc.scalar.dma_start(xt[:], io["xT"][ts(din, P), :])
        xt_tiles.append(xt)

    def project_tmajor(wT_ap, bias_ap, out_tiles, dma_eng, pairs=None):
        """out[t, dout] = x @ W^T + b, token-major tiles [128, d_model]."""
        for pair in (range(CHUNKS // PAIR) if pairs is None else pairs):
            psums = [psum(f"pp{i}", N_CHUNK) for i in range(T_TILES * PAIR)]
            for din in range(DIN):
                wt = b4(f"w{din}")
                dma_eng.dma_start(wt[:], wT_ap[ts(din, P),
                                               ts(pair, PAIR * N_CHUNK)])
                for t in range(T_TILES):
                    for p in range(PAIR):
                        nc.tensor.matmul(
                            psums[t * PAIR + p][:],
                            xt_tiles[din][:, ts(t, P)],
                            wt[:, ts(p, N_CHUNK)],
                            start=(din == 0),
                            stop=(din == DIN - 1),
                        )
            bt = b4(f"bias_p{pair}")
            dma_eng.dma_start(bt[:], bias_ap[:, ts(pair, PAIR * N_CHUNK)])
            for t in range(T_TILES):
                for p in range(PAIR):
                    chunk = pair * PAIR + p
                    nc.vector.tensor_add(
                        out_tiles[t][:, ts(chunk, N_CHUNK)],
                        psums[t * PAIR + p][:],
                        bt[:, ts(p, N_CHUNK)],
                    )

    def rope_tmajor(tiles, cos_ap, sinf_ap, t_range=None):
        """In-place RoPE on token-major tiles using compact [t, HD] tables
        broadcast across heads.

        out = x*cos + rot_half(x)*sin; sinf is sign-folded so
        rot_half(x)*sin == gather(x, +-64) * sinf elementwise.
        """
        h2 = HD // 2

        def bc(ap2d):  # [128, w] -> [128, NH, w] broadcast view
            return ap2d.unsqueeze(1).broadcast_to([P, NH, ap2d.shape[-1]])

        for t in (range(T_TILES) if t_range is None else t_range):
            ct = b2(f"cos{t}", width=HD)
            st = b2(f"sin{t}", width=HD)
            nc.scalar.dma_start(ct[:], cos_ap[ts(t, P), :])
            nc.scalar.dma_start(st[:], sinf_ap[ts(t, P), :])
            tmp = b8(f"ropetmp{t}")
            x3 = tiles[t][:].rearrange("p (h d) -> p h d", d=HD)
            t3 = tmp[:].rearrange("p (h d) -> p h d", d=HD)
            nc.vector.tensor_mul(t3[:, :, 0:h2], x3[:, :, h2:HD],
                                 bc(st[:, 0:h2]))
            nc.vector.tensor_mul(t3[:, :, h2:HD], x3[:, :, 0:h2],
                                 bc(st[:, h2:HD]))
            nc.vector.tensor_mul(x3, x3, bc(ct[:]))
            nc.vector.tensor_add(tiles[t][:], tiles[t][:], tmp[:])

    # ---- K/V projections + RoPE(k) ----
    k_tiles = [b8(f"k{t}") for t in range(T_TILES)]
    project_tmajor(io["wkT"], io["bkb"], k_tiles, nc.sync)
    rope_tmajor(k_tiles, io["cosk"], io["sinkf"])
    v_tiles = [b8(f"v{t}") for t in range(T_TILES)]
    project_tmajor(io["wvT"], io["bvb"], v_tiles, nc.scalar)

    # ---- Per-head Gram matrices KV[h] = k_h^T @ v_h (partial over this
    # core's tokens), packed [128, NH*HD]. The moving operand spans two
    # heads (N=256) to stay at full fp32r rate; the unwanted half of each
    # product is discarded at eviction. ----
    kv_sb = b8("kvsb")

    def kv_partials(h_lo, h_hi):
        for h in range(h_lo, h_hi):
            base = min(h, NH - 2) if NH >= 2 else 0
            good = h - base
            width = min(2 * HD, (NH - base) * HD)
            kvp = psum(f"kvp{h}", width)
            for t in range(T_TILES):
                nc.tensor.matmul(
                    kvp[:],
                    k_tiles[t][:, ts(h, HD)],
                    v_tiles[t][:, base * HD: base * HD + width],
                    start=(t == 0),
                    stop=(t == T_TILES - 1),
                )
            nc.vector.tensor_copy(kv_sb[:, ts(h, HD)], kvp[:, ts(good, HD)])

    # ---- AllReduce the KV partials within the batch group, in two halves
    # so attention on the first heads can start while the second half is
    # still reducing ----
    kv_red = b8("kvred")
    kv_perm = b8("kvperm")
    h2 = HD // 2
    N_SPLIT = 2 if NH >= 2 else 1
    HS = NH // N_SPLIT            # heads per collective
    W_HALF = HS * HD
    for s in range(N_SPLIT):
        kv_partials(s * HS, (s + 1) * HS)
        kv_in = dram.tile([P, W_HALF], F32R, name=f"kv_in{s}")
        kv_out = dram.tile([P, W_HALF], F32R, name=f"kv_out{s}")
        nc.gpsimd.dma_start(kv_in[:], kv_sb[:, ts(s, W_HALF)])
        nc.gpsimd.collective_compute(
            "AllReduce",
            mybir.AluOpType.add,
            replica_groups=replica_groups,
            ins=[kv_in.opt()],
            outs=[kv_out.opt()],
        )
        nc.gpsimd.dma_start(kv_red[:, ts(s, W_HALF)], kv_out[:])
        # Partition-half-swapped copy for the folded q-side RoPE
        nc.gpsimd.dma_start(kv_perm[0:h2, ts(s, W_HALF)], kv_out[h2:HD, :])
        nc.gpsimd.dma_start(kv_perm[h2:HD, ts(s, W_HALF)], kv_out[0:h2, :])

    # ---- Q projection, feature-major: qD[dout, t] = W q-row blocks ----
    qd_tiles = [None] * DIN
    qcs = [None] * DIN
    bqd_sb = b2("bqd", dtype=F32, width=DIN)
    nc.sync.dma_start(bqd_sb[:], io["bqd"][:])
    cosq = b2("cosq", width=t_core)
    sinq = b2("sinq", width=t_core)
    nc.scalar.dma_start(cosq[:], io["cosqD"][:])
    nc.scalar.dma_start(sinq[:], io["sinqD"][:])
    for g in range(DIN // QG):
        psums = [psum(f"qp{i}", t_core) for i in range(QG)]
        for din in range(DIN):
            wt = b4(f"wq{din}", width=QG * P)
            nc.sync.dma_start(wt[:], io["wqT"][ts(din, P),
                                               ts(g, QG * P)])
            for j in range(QG):
                nc.tensor.matmul(
                    psums[j][:],
                    wt[:, ts(j, P)],
                    xt_tiles[din][:],
                    start=(din == 0),
                    stop=(din == DIN - 1),
                )
        for j in range(QG):
            dout = g * QG + j
            qd = b2(f"qd{dout}", width=t_core)
            nc.vector.tensor_scalar_add(qd[:], psums[j][:],
                                        bqd_sb[:, dout:dout + 1])
            qd_tiles[dout] = qd
            # RoPE multiplies hoisted here so they overlap the collective
            # wait; the attention phase is then pure PE.
            qc = b2(f"qc{dout}", width=t_core)
            nc.vector.tensor_mul(qc[:], qd[:], cosq[:])
            qs = b2(f"qs{dout}", width=t_core)
            nc.vector.tensor_mul(qs[:], qd[:], sinq[:])
            qcs[dout] = (qc, qs)

    # ---- attn_h[d2, t] = KV_h^T (cos*q_h) + KVp_h^T (sin* q_h) ----
    attn_tiles = []
    for h in range(NH):
        qc, qs = qcs[h]
        ap = psum(f"ap{h}", t_core)
        nc.tensor.matmul(ap[:], kv_red[:, ts(h, HD)], qc[:],
                         start=True, stop=False)
        nc.tensor.matmul(ap[:], kv_perm[:, ts(h, HD)], qs[:],
                         start=False, stop=True)
        asb = b2(f"asb{h}", width=t_core)
        nc.vector.tensor_copy(asb[:], ap[:])
        attn_tiles.append(asb)

    # ---- Output projection: y[t, dout] = attn @ Wo^T, token-major ----
    for pair in range(CHUNKS // PAIR):
        psums = [psum(f"op{i}", N_CHUNK) for i in range(T_TILES * PAIR)]
        for dmid in range(DIN):
            wt = b4(f"wo{dmid}")
            nc.scalar.dma_start(wt[:], io["woT"][ts(dmid, P),
                                                 ts(pair, PAIR * N_CHUNK)])
            for t in range(T_TILES):
                for p in range(PAIR):
                    nc.tensor.matmul(
                        psums[t * PAIR + p][:],
                        attn_tiles[dmid][:, ts(t, P)],
                        wt[:, ts(p, N_CHUNK)],
                        start=(dmid == 0),
                        stop=(dmid == DIN - 1),
                    )
        for t in range(T_TILES):
            for p in range(PAIR):
                chunk = pair * PAIR + p
                osb = b2(f"osb{chunk}_{t}", dtype=F32)
                nc.vector.tensor_copy(osb[:], psums[t * PAIR + p][:])
                nc.gpsimd.dma_start(io["y"][ts(t, P), ts(chunk, N_CHUNK)],
                                    osb[:])


def build_nc(t_core, d_model, num_devices, replica_groups, reps=1):
    nc = bacc.Bacc("TRN2", target_bir_lowering=False, debug=False,
                   num_devices=num_devices)
    io = {}
    io["xT"] = nc.dram_tensor("xT", [d_model, t_core], F32R,
                              kind="ExternalInput").ap()
    for nm in ("wqT", "wkT", "wvT", "woT"):
        io[nm] = nc.dram_tensor(nm, [d_model, d_model], F32R,
                                kind="ExternalInput").ap()
    for nm in ("bkb", "bvb"):
        io[nm] = nc.dram_tensor(nm, [P, d_model], F32R,
                                kind="ExternalInput").ap()
    io["bqd"] = nc.dram_tensor("bqd", [P, d_model // P], F32,
                               kind="ExternalInput").ap()
    for nm in ("cosk", "sinkf"):
        io[nm] = nc.dram_tensor(nm, [t_core, HD], F32R,
                                kind="ExternalInput").ap()
    for nm in ("cosqD", "sinqD"):
        io[nm] = nc.dram_tensor(nm, [P, t_core], F32R,
                                kind="ExternalInput").ap()
    io["y"] = nc.dram_tensor("y", [t_core, d_model], F32,
                             kind="ExternalOutput").ap()

    with tile.TileContext(nc) as tc:
        for _ in range(reps):
            with ExitStack() as ctx:
                emit_attn(tc, ctx, io, t_core, d_model, replica_groups)
    nc.compile()
    return nc


# ---------------- host side ----------------

B, S, D = 2, 2048, 2048
NH_FULL = 16
MAX_POS = 4096
ROPE_THETA = 10000.0
N_CORES = 8
T_CORE = B * S // N_CORES

_cache = {}


def _rope_tables():
    inv_freq = (np.float32(1.0) /
                np.power(np.float32(ROPE_THETA),
                         np.arange(0, HD, 2, dtype=np.float32) /
                         np.float32(HD))).astype(np.float32)
    t = np.arange(MAX_POS, dtype=np.float32)
    freqs = np.outer(t, inv_freq).astype(np.float32)
    emb = np.concatenate((freqs, freqs), axis=-1)
    return np.cos(emb).astype(np.float32), np.sin(emb).astype(np.float32)


def _get_nc():
    if "nc" not in _cache:
        _cache["nc"] = build_nc(T_CORE, D, N_CORES,
                                [[0, 1, 2, 3], [4, 5, 6, 7]])
    return _cache["nc"]


def _host_inputs(hidden_states, position_ids, Wq, bq, Wk, bk, Wv, bv, Wo):
    x = np.asarray(hidden_states, dtype=np.float32).reshape(B * S, D)
    pos = np.asarray(position_ids).astype(np.int64).reshape(B * S)

    cos_t, sin_t = _rope_tables()
    cos = cos_t[pos]            # [B*S, HD]
    sin = sin_t[pos]
    # token-major k tables: sign-folded sin + 1/sqrt(HD) fold
    sinf = sin.copy()
    sinf[:, : HD // 2] *= np.float32(-1.0)
    scale = np.float32(1.0 / math.sqrt(HD))
    # feature-major q tables: sin* = +sin (i<64), -sin (i>=64)
    sinq = sin.copy()
    sinq[:, HD // 2:] *= np.float32(-1.0)

    wqT = np.ascontiguousarray(np.asarray(Wq, np.float32).T)
    wkT = np.ascontiguousarray(np.asarray(Wk, np.float32).T)
    wvT = np.ascontiguousarray(np.asarray(Wv, np.float32).T)
    woT = np.ascontiguousarray(np.asarray(Wo, np.float32).T)
    bkb = np.ascontiguousarray(np.broadcast_to(np.asarray(bk, np.float32), (P, D)))
    bvb = np.ascontiguousarray(np.broadcast_to(np.asarray(bv, np.float32), (P, D)))
    bqd = np.ascontiguousarray(np.asarray(bq, np.float32).reshape(D // P, P).T)

    in_maps = []
    for c in range(N_CORES):
        sl = slice(c * T_CORE, (c + 1) * T_CORE)
        in_maps.append({
            "xT": np.ascontiguousarray(x[sl].T),
            "wqT": wqT, "wkT": wkT, "wvT": wvT, "woT": woT,
            "bkb": bkb, "bvb": bvb, "bqd": bqd,
            "cosk": np.ascontiguousarray(cos[sl] * scale),
            "sinkf": np.ascontiguousarray(sinf[sl] * scale),
            "cosqD": np.ascontiguousarray(cos[sl].T),
            "sinqD": np.ascontiguousarray(sinq[sl].T),
        })
    return in_maps


def kernel(hidden_states, position_ids, Wq, bq, Wk, bk, Wv, bv, Wo):
    in_maps = _host_inputs(hidden_states, position_ids,
                           Wq, bq, Wk, bk, Wv, bv, Wo)
    nc = _get_nc()
    last_err = None
    for attempt in range(3):
        try:
            res = bass_utils.run_bass_kernel_spmd(
                nc, in_maps, core_ids=list(range(N_CORES)))
            break
        except Exception as e:  # transient axon/device states clear on retry
            last_err = e
            import time
            time.sleep(15 * (attempt + 1))
    else:
        raise last_err
    out = np.concatenate([res.results[c]["y"] for c in range(N_CORES)], axis=0)
    return out.reshape(B, S, D)
